# revision 1
# baseline (speedup 1.0000x reference)
"""Causal self-attention (B=4, S=2048, D=1024, H=16) on 8 Trainium2 cores.

Sharding: core c handles batch b = c//2 and head-group g = c%2 (8 heads).
Each core computes q/k/v projections for its head group, causal attention
over its 8 heads, and a partial output projection over its 512 columns of
Wo. The host sums the two per-batch partials and adds bo.

Device layouts (per core):
  x_T   [d=1024 (8x128 part-tiles), s=2048]          (stationary for v-proj, moving for q/k-proj)
  q_T,k_T [e=512 (4 pair-tiles of 128 = 2 heads), s] (partition = head dim)
  v_aug [s (16 tiles of 128), 8 heads x (64 v + ones col)]
  scores_T = k_h^T-stationary matmul -> [j (keys) down, i (queries) free] in PSUM
  P = exp(scale * scores_T) in bf16, causal-masked by 0/1 mask mul
  y~, denom = v_aug^T @ P (ones column gives the softmax denominator)
  y = y~ * bcast(1/denom); out_T[o down, s] = Wo_g^T-stationary matmul over e
"""

import numpy as np
import ml_dtypes

import concourse.bass as bass
import concourse.mybir as mybir
import concourse.tile as tile
from concourse.vector_clock import ScopedClock

B, S, D, H = 4, 2048, 1024, 16
HD = D // H  # 64
N_CORES = 8
HG = 2  # head groups (cores per batch)
EG = D // HG  # 512 e-columns per core
PAIRS = 4  # head pairs per core
DC = D // 128  # 8 d-chunks
SC = S // 512  # 4 s/i chunks
SBLK = S // 128  # 16 s blocks
SCALE = 1.0 / float(np.sqrt(HD))

F32 = mybir.dt.float32
MM_DT = mybir.dt.bfloat16
NP_MM_DT = ml_dtypes.bfloat16

AF = mybir.ActivationFunctionType


# ---------------------------------------------------------------------------
# Workaround: this walrus build allows only ONE sync-wait on a CTRL (Drain)
# instruction; TileContext's tail drain attaches one wait per active logical
# processor. Split them across a chain of Drains (same SP program order).
def _split_drain_and_barrier(self, tick_clock, wait_clock):
    drain_inst = self.nc.sync.drain()
    wait_clock.add_sem_waits(
        drain_inst.ins, ScopedClock({None: tick_clock.global_clock})
    )
    si = drain_inst.ins.sync_info
    waits = list(si.on_wait) if si is not None and si.on_wait else []
    if len(waits) > 1:
        si.on_wait = waits[:1]
        for w in waits[1:]:
            extra = self.nc.sync.drain()
            esi = extra.ins.sync_info
            if esi is None:
                extra.ins.sync_info = mybir.SyncInfo(on_wait=[w], on_update=[])
            else:
                esi.on_wait = [w]
    self.nc.all_engine_barrier()
    assert self.sems is not None
    popped = self.nc._tile_sem_poison_stack.pop()
    assert popped is self._sem_poison
    self.nc.clear_and_free_semaphores(list(self.sems.allocated().values()))
    self.nc.all_engine_barrier()


tile.TileContext._drain_and_barrier = _split_drain_and_barrier


# The same 1-wait limit applies to every instruction class, so split any
# multi-wait instruction by hoisting excess waits onto same-engine no-fuse
# NOPs committed immediately before it (identical semantics: the engine
# executes them in order before the instruction issues).
_orig_commit = tile.TileContext._commit_instruction


def _split_commit(self, inst, lazy_reg_writes=True):
    si = getattr(inst, "sync_info", None)
    if (
        si is not None
        and si.on_wait
        and len(si.on_wait) > 1
        and getattr(inst, "engine", mybir.EngineType.Unassigned)
        != mybir.EngineType.Unassigned
    ):
        waits = list(si.on_wait)
        si.on_wait = waits[-1:]
        for w in waits[:-1]:
            nop = mybir.InstNoOp(
                name=self.nc.get_next_instruction_name(),
                engine=inst.engine,
                bass_nofuse=True,
                sync_info=mybir.SyncInfo(on_wait=[w], on_update=[]),
            )
            _orig_commit(self, nop, lazy_reg_writes=False)
    return _orig_commit(self, inst, lazy_reg_writes=lazy_reg_writes)


tile.TileContext._commit_instruction = _split_commit
# ---------------------------------------------------------------------------


def build_program():
    nc = bass.Bass()

    xT = nc.dram_tensor("xT", [DC, 128, S], MM_DT, kind="ExternalInput")
    wqT = nc.dram_tensor("wqT", [DC, 128, EG], MM_DT, kind="ExternalInput")
    wkT = nc.dram_tensor("wkT", [DC, 128, EG], MM_DT, kind="ExternalInput")
    wvT = nc.dram_tensor("wvT", [DC, 128, EG], MM_DT, kind="ExternalInput")
    woT = nc.dram_tensor("woT", [PAIRS, 128, D], MM_DT, kind="ExternalInput")
    bqd = nc.dram_tensor("bqd", [128, PAIRS], F32, kind="ExternalInput")
    bkd = nc.dram_tensor("bkd", [128, PAIRS], F32, kind="ExternalInput")
    bvaug = nc.dram_tensor("bvaug", [520], F32, kind="ExternalInput")
    outd = nc.dram_tensor("out", [DC, 128, S], F32, kind="ExternalOutput")

    with tile.TileContext(nc) as tc:
        with (
            tc.tile_pool(name="const", bufs=1) as const,
            tc.tile_pool(name="big", bufs=1) as big,
            tc.tile_pool(name="ppool", bufs=6) as ppool,
            tc.tile_pool(name="ytil", bufs=10) as ytil,
            tc.tile_pool(name="dnp", bufs=4) as dnp,
            tc.tile_pool(name="rcp", bufs=4) as rcp,
            tc.tile_pool(name="bcp", bufs=6) as bcp,
            tc.tile_pool(name="drp", bufs=6, space="DRAM") as drp,
            tc.tile_pool(name="ostg", bufs=4) as ostg,
            tc.tile_pool(name="pj", bufs=2, space="PSUM") as pj,
            tc.tile_pool(name="scps", bufs=2, space="PSUM") as scps,
            tc.tile_pool(name="yps", bufs=2, space="PSUM") as yps,
        ):
            # ---- persistent SBUF tensors
            x_sb = big.tile([128, DC, S], MM_DT)
            wq_sb = const.tile([128, DC, EG], MM_DT)
            wk_sb = const.tile([128, DC, EG], MM_DT)
            wv_sb = const.tile([128, DC, EG], MM_DT)
            wo_sb = const.tile([128, PAIRS, D], MM_DT)
            bqs = const.tile([128, PAIRS], F32)
            bks = const.tile([128, PAIRS], F32)
            bvb = const.tile([128, 520], F32)
            q_sb = big.tile([128, PAIRS, S], MM_DT)
            k_sb = big.tile([128, PAIRS, S], MM_DT)
            v_sb = big.tile([128, SBLK, 520], MM_DT)
            y_sb = big.tile([128, PAIRS, S], MM_DT)

            # ---- input loads (wq+wk+x first: q/k projections gate attention)
            for dc in range(DC):
                nc.sync.dma_start(wq_sb[:, dc, :], wqT[dc])
                nc.sync.dma_start(wk_sb[:, dc, :], wkT[dc])
                nc.sync.dma_start(x_sb[:, dc, :], xT[dc])
            for dc in range(DC):
                nc.sync.dma_start(wv_sb[:, dc, :], wvT[dc])
            for ec in range(PAIRS):
                nc.sync.dma_start(wo_sb[:, ec, :], woT[ec])
            nc.sync.dma_start(bqs[:], bqd[:])
            nc.sync.dma_start(bks[:], bkd[:])
            bv_ap = bvaug[:]
            nc.sync.dma_start(
                bvb[:],
                bass.AP(tensor=bv_ap.tensor, offset=bv_ap.offset,
                        ap=[[0, 128]] + list(bv_ap.ap)),
            )
            # ones columns of v_aug (col 64 of each head slot)
            v4 = v_sb[:].rearrange("p t (h c) -> p t h c", h=8)
            nc.vector.memset(v4[:, :, :, 64:65], 1.0)

            def emit_proj(sc):
                ssl = slice(512 * sc, 512 * (sc + 1))
                for pair in range(PAIRS):
                    psl = slice(128 * pair, 128 * (pair + 1))
                    for nm, w_sb, dst, bias in (
                        ("q", wq_sb, q_sb, bqs),
                        ("k", wk_sb, k_sb, bks),
                    ):
                        pq = pj.tile([128, 512], F32, tag="pj", name=f"p{nm}{sc}_{pair}")
                        for dc in range(DC):
                            nc.tensor.matmul(
                                pq[:],
                                w_sb[:, dc, psl],
                                x_sb[:, dc, ssl],
                                start=(dc == 0),
                                stop=(dc == DC - 1),
                            )
                        nc.vector.tensor_scalar_add(
                            dst[:, pair, ssl], pq[:], bias[:, pair : pair + 1]
                        )
                for sb_i in range(4 * sc, 4 * sc + 4):
                    pv = pj.tile([128, 512], F32, tag="pj", name=f"pv{sb_i}")
                    for dc in range(DC):
                        nc.tensor.matmul(
                            pv[:],
                            x_sb[:, dc, 128 * sb_i : 128 * (sb_i + 1)],
                            wv_sb[:, dc, :],
                            start=(dc == 0),
                            stop=(dc == DC - 1),
                        )
                    nc.vector.tensor_add(
                        v4[:, sb_i, :, 0:64],
                        pv[:].rearrange("p (h c) -> p h c", h=8),
                        bvb[:].rearrange("p (h c) -> p h c", h=8)[:, :, 0:64],
                    )

            def emit_attn_pair(c, pair, dn_half):
                """QK/exp/mask/PV for one head pair; returns the two yt tiles."""
                isl = slice(512 * c, 512 * (c + 1))
                ypsum = [
                    yps.tile([65, 512], F32, name=f"ypsum{c}_{pair}_{a}", tag="yp")
                    for a in range(2)
                ]
                n_jb = 4 * c + 4
                n_jg = n_jb // 2
                for jg in range(n_jg):
                    b0 = 2 * jg
                    pss = [
                        scps.tile([128, 1024], F32, name=f"ps{c}_{pair}_{jg}_{a}", tag="ps_s")
                        for a in range(2)
                    ]
                    pts = [
                        ppool.tile([128, 1024], MM_DT, name=f"pt{c}_{pair}_{jg}_{a}", tag="pt")
                        for a in range(2)
                    ]
                    # interleave A/B so adjacent matmuls hit disjoint PE row groups
                    for bi in range(2):
                        b = b0 + bi
                        for a in range(2):
                            p0, p1 = 64 * a, 64 * a + 64
                            nc.tensor.matmul(
                                pss[a][:, 512 * bi : 512 * (bi + 1)],
                                k_sb[p0:p1, pair, 128 * b : 128 * (b + 1)],
                                q_sb[p0:p1, pair, isl],
                            )
                    for a in range(2):
                        nc.scalar.activation(pts[a][:], pss[a][:], AF.Exp, scale=SCALE)
                        for bi in range(2):
                            off = b0 + bi - 4 * c
                            if off >= 0:
                                base_col = 512 * bi
                                if off > 0:
                                    # columns i < 128*off are fully masked
                                    nc.gpsimd.memset(
                                        pts[a][:, base_col : base_col + 128 * off], 0.0
                                    )
                                tri = slice(base_col + 128 * off, base_col + 128 * off + 128)
                                # triangle: keep where i - j >= 0 (i local to this 128-col window)
                                nc.gpsimd.affine_select(
                                    out=pts[a][:, tri],
                                    in_=pts[a][:, tri],
                                    pattern=[[1, 128]],
                                    compare_op=mybir.AluOpType.is_ge,
                                    fill=0.0,
                                    base=0,
                                    channel_multiplier=-1,
                                )
                    for a in range(2):
                        hh = 2 * pair + a
                        hsl = slice(65 * hh, 65 * (hh + 1))
                        for bi in range(2):
                            nc.tensor.matmul(
                                ypsum[a][:],
                                v_sb[:, b0 + bi, hsl],
                                pts[a][:, 512 * bi : 512 * (bi + 1)],
                                start=(jg == 0 and bi == 0),
                                stop=(jg == n_jg - 1 and bi == 1),
                            )
                yts = []
                for a in range(2):
                    yt = ytil.tile([65, 512], MM_DT, name=f"yt{c}_{pair}_{a}", tag="yt")
                    nc.scalar.copy(yt[:], ypsum[a][:])
                    nc.sync.dma_start(
                        dn_half[(2 * pair + a) % 4 : (2 * pair + a) % 4 + 1, :],
                        yt[64:65, :],
                    )
                    yts.append(yt)
                return yts

            def emit_norm_half(c, half, yts4, dn_half):
                """reciprocal + broadcast + scale for pairs (2*half, 2*half+1)."""
                isl = slice(512 * c, 512 * (c + 1))
                rc = rcp.tile([4, 512], F32, name=f"rc{c}_{half}", tag="rc")
                nc.vector.reciprocal(rc[:], dn_half[:])
                for k_i, yt in enumerate(yts4):
                    pair = 2 * half + k_i // 2
                    a = k_i % 2
                    scr = drp.tile([1, 512], F32, name=f"scr{c}_{2*pair+a}", tag="scr")
                    nc.sync.dma_start(scr[:], rc[k_i : k_i + 1, :])
                    bc = bcp.tile([64, 512], F32, name=f"bc{c}_{2*pair+a}", tag="bc")
                    scr_ap = scr[:]
                    nc.sync.dma_start(
                        bc[:],
                        bass.AP(tensor=scr_ap.tensor, offset=scr_ap.offset,
                                ap=[[0, 64]] + list(scr_ap.ap[1:])),
                    )
                    nc.vector.tensor_mul(
                        y_sb[64 * a : 64 * a + 64, pair, isl],
                        yt[0:64, :],
                        bc[:],
                    )

            def emit_oproj(c):
                isl = slice(512 * c, 512 * (c + 1))
                for ob in range(DC):
                    po = scps.tile([128, 512], F32, tag="ps_s", name=f"po{c}_{ob}")
                    for ec in range(PAIRS):
                        nc.tensor.matmul(
                            po[:],
                            wo_sb[:, ec, 128 * ob : 128 * (ob + 1)],
                            y_sb[:, ec, isl],
                            start=(ec == 0),
                            stop=(ec == PAIRS - 1),
                        )
                    ost = ostg.tile([128, 512], F32, name=f"ost{c}_{ob}", tag="ost")
                    nc.vector.tensor_copy(ost[:], po[:])
                    nc.sync.dma_start(outd[ob, :, isl], ost[:])

            pending_oproj = None
            for sc in range(SC):
                emit_proj(sc)
                for half in range(2):
                    dn_half = dnp.tile([4, 512], MM_DT, name=f"dn{sc}_{half}", tag="dn")
                    yts4 = []
                    for pair in (2 * half, 2 * half + 1):
                        yts4 += emit_attn_pair(sc, pair, dn_half)
                        if pending_oproj is not None:
                            emit_oproj(pending_oproj)
                            pending_oproj = None
                    emit_norm_half(sc, half, yts4, dn_half)
                pending_oproj = sc
            emit_oproj(pending_oproj)

    return nc


def make_core_inputs(x, Wq, bq, Wk, bk, Wv, bv, Wo, bo, core):
    b, g = core // HG, core % HG
    esl = slice(EG * g, EG * (g + 1))
    xT = np.ascontiguousarray(x[b].T).reshape(DC, 128, S)
    wqT = np.ascontiguousarray(Wq[esl, :].T).reshape(DC, 128, EG)
    wkT = np.ascontiguousarray(Wk[esl, :].T).reshape(DC, 128, EG)
    wvT = np.ascontiguousarray(Wv[esl, :].T).reshape(DC, 128, EG)
    woT = np.ascontiguousarray(Wo[:, esl].T).reshape(PAIRS, 128, D)
    bqd = np.ascontiguousarray(bq[esl].reshape(PAIRS, 128).T)
    bkd = np.ascontiguousarray(bk[esl].reshape(PAIRS, 128).T)
    bvaug = np.zeros(520, dtype=np.float32)
    for h in range(8):
        bvaug[65 * h : 65 * h + 64] = bv[EG * g + 64 * h : EG * g + 64 * (h + 1)]
    return {
        "xT": xT.astype(NP_MM_DT),
        "wqT": wqT.astype(NP_MM_DT),
        "wkT": wkT.astype(NP_MM_DT),
        "wvT": wvT.astype(NP_MM_DT),
        "woT": woT.astype(NP_MM_DT),
        "bqd": bqd.astype(np.float32),
        "bkd": bkd.astype(np.float32),
        "bvaug": bvaug,
    }


_CACHED_NC = None


def kernel(x, Wq, bq, Wk, bk, Wv, bv, Wo, bo):
    global _CACHED_NC
    from concourse.bass_utils import run_bass_kernel_spmd

    x = np.asarray(x, dtype=np.float32)
    Wq = np.asarray(Wq, dtype=np.float32)
    bq = np.asarray(bq, dtype=np.float32)
    Wk = np.asarray(Wk, dtype=np.float32)
    bk = np.asarray(bk, dtype=np.float32)
    Wv = np.asarray(Wv, dtype=np.float32)
    bv = np.asarray(bv, dtype=np.float32)
    Wo = np.asarray(Wo, dtype=np.float32)
    bo = np.asarray(bo, dtype=np.float32)

    if _CACHED_NC is None:
        _CACHED_NC = build_program()
    nc = _CACHED_NC

    in_maps = [
        make_core_inputs(x, Wq, bq, Wk, bk, Wv, bv, Wo, bo, core)
        for core in range(N_CORES)
    ]
    res = run_bass_kernel_spmd(nc, in_maps, list(range(N_CORES)))

    out = np.empty((B, S, D), dtype=np.float32)
    for b in range(B):
        o0 = res.results[HG * b]["out"].reshape(D, S)
        o1 = res.results[HG * b + 1]["out"].reshape(D, S)
        out[b] = (o0 + o1).T + bo[None, :]
    return out

